# revision 10
# baseline (speedup 1.0000x reference)
"""CapsuleLinear (dynamic routing) Trainium2 kernel — Gram-matrix formulation.

Reference computes priors = einsum('oli,bni->bonl', W, x) then 3 routing
iterations. We never materialize priors. Per routing iteration r:
    probs[n,o] = softmax_o(logits[n,o])        exp on ACT, Z on DVE
    s[o,i]     = sum_n probs[n,o] x[n,i]       PE matmul (contract n)
    gs[o,i]    = G[o] @ s[o]    G = W^T W      DVE mul+reduce (block-diag)
    ns[o]      = dot(s[o], gs[o]) = ||W s||^2  DVE mul + ACT accum
    factor[o]  = sqrt(ns)/(1+ns)               ACT ln/exp + DVE
    wv[o,i]    = factor * gs[o,i]              DVE tensor_scalar (tiny)
    wv_cum    += wv;  logits = x @ wv_cum^T    PE matmuls, fresh PSUM group
Final (r=2): v = factor * (W_li s) computed the direct way.

The Gram trick kills the old wv-step broadcast-mul+reduce entirely:
wv = factor*gs reuses the gs already needed for ns, and the out-step
(W_li s) only runs once, at r=2.

Iter 0 shortcut: probs are uniform (logits=0), so s0 = sum_n(x)/64 is
computed on host (tiny reduction) and DMA'd straight into s_sb: iteration
0 runs gs -> squash -> wv -> delta with no softmax/s-matmul at all.

Scheduling: pair 0 (batches 0,1) is the latency-critical chain each
iteration — its delta matmuls (h0) feed the next iteration's first exp.
Pair 1 trails at a constant offset. Emission order per engine queue keeps
pair-0 dependencies first.

Sharding: data-parallel over batch N=32 -> 4 per core on 8 cores, weight
and Gram replicated.  No collectives.
"""

import os
import sys

for _p in ("/opt/trn_rl_repo",):
    if _p not in sys.path and os.path.isdir(_p):
        sys.path.insert(0, _p)

import numpy as np

import concourse.bacc as bacc
import concourse.bass as bass
import concourse.tile as tile
from concourse import mybir
from concourse.bass_utils import run_bass_kernel_spmd

N_TOT, N_CAPS, I_LEN = 32, 1152, 32
O_CAPS, L_LEN = 64, 32
NCORES = 8
B = N_TOT // NCORES  # 4 batches per core
C = N_CAPS // 128    # 9 chunks of 128 input capsules
PAIRS = B // 2
FP = mybir.dt.float32
BF = mybir.dt.bfloat16
Exp = mybir.ActivationFunctionType.Exp
Ln = mybir.ActivationFunctionType.Ln
Square = mybir.ActivationFunctionType.Square
Copy = mybir.ActivationFunctionType.Copy
X = mybir.AxisListType.X
MUL = mybir.AluOpType.mult


def build_nc():
    nc = bacc.Bacc("TRN2", target_bir_lowering=False, debug=True)
    x_nat_d = nc.dram_tensor("x_nat", [128, B, C, I_LEN], BF, kind="ExternalInput")
    xt_d = nc.dram_tensor("xt", [I_LEN, B, C, 128], BF, kind="ExternalInput")
    g_d = nc.dram_tensor("g", [128, I_LEN, I_LEN], BF, kind="ExternalInput")
    w_li_d = nc.dram_tensor("w_li", [64, L_LEN, I_LEN], BF, kind="ExternalInput")
    s0_d = nc.dram_tensor("s0", [128, PAIRS, I_LEN], BF, kind="ExternalInput")
    ident_d = nc.dram_tensor("ident", [128, 128], FP, kind="ExternalInput")
    out_d = nc.dram_tensor("out", [PAIRS, 128, L_LEN], FP, kind="ExternalOutput")

    with tile.TileContext(nc) as tc:
        with (
            tc.tile_pool(name="main", bufs=1) as pool,
            tc.tile_pool(name="psum", bufs=1, space="PSUM") as psum,
        ):
            x_sb = pool.tile([128, B, C, I_LEN], BF)
            xt_sb = pool.tile([I_LEN, B, C, 128], BF)
            g_sb = pool.tile([128, I_LEN, I_LEN], BF)
            wli_sb = pool.tile([128, L_LEN, I_LEN], BF)
            ident = pool.tile([128, 128], FP)
            shift = pool.tile([128, 1], FP)
            pexp = pool.tile([128, B, C, O_CAPS], BF)
            zsum = pool.tile([128, B, C], FP)
            rinv = pool.tile([128, B, C], FP)
            xr = pool.tile([128, B, C, I_LEN], BF)
            s_sb = pool.tile([128, PAIRS, I_LEN], BF)
            gprod = pool.tile([128, PAIRS, I_LEN, I_LEN], BF)
            gs = pool.tile([128, PAIRS, I_LEN], FP)
            dotp = pool.tile([128, PAIRS, I_LEN], FP)
            ns = pool.tile([128, PAIRS], FP)
            lnns = pool.tile([128, PAIRS], FP)
            vnorm = pool.tile([128, PAIRS], FP)
            denom = pool.tile([128, PAIRS], FP)
            rden = pool.tile([128, PAIRS], FP)
            wv_new = pool.tile([128, PAIRS, I_LEN], FP)
            wvt_sb = pool.tile([I_LEN, PAIRS, 128], BF)
            # r=2 final out-step
            prod = pool.tile([128, PAIRS, L_LEN, I_LEN], BF)
            v_raw = pool.tile([128, PAIRS, L_LEN], FP)
            sq = pool.tile([128, PAIRS, L_LEN], FP)
            v = pool.tile([128, PAIRS, L_LEN], FP)

            logits_ps = [
                psum.tile([128, 2, C, O_CAPS], FP, name=f"logits_ps{h}", tag=f"lg{h}")
                for h in range(2)
            ]
            # s (bytes 0..127) and wvT (bytes 512..1023) share a bank per pair
            u_ps = [
                psum.tile([128, 512], FP, name=f"u_ps{t}", tag=f"u_ps{t}")
                for t in range(PAIRS)
            ]
            s_ps = [u_ps[t][:, 0:I_LEN] for t in range(PAIRS)]
            wvt_ps = [u_ps[t][0:I_LEN, 128:256] for t in range(PAIRS)]

            # s0+g first on the sync queue (they unblock iteration 0's
            # gs-path), x third; xt/ident on the gpsimd queue; wli (only
            # needed at r=2) on the scalar queue.
            nc.sync.dma_start(out=s_sb[:], in_=s0_d[:])
            nc.sync.dma_start(out=g_sb[:], in_=g_d[:])
            with tc.tile_wait_until(0.0015):
                nc.gpsimd.dma_start(out=xt_sb[:], in_=xt_d[:])
                nc.gpsimd.dma_start(out=ident[:], in_=ident_d[:])
            with tc.tile_wait_until(0.002):
                nc.sync.dma_start(out=x_sb[:], in_=x_nat_d[:])
            with tc.tile_wait_until(0.003):
                nc.scalar.dma_start(out=wli_sb[0:64], in_=w_li_d[:])
                nc.scalar.dma_start(out=wli_sb[64:128], in_=wli_sb[0:64])
            nc.vector.memset(shift[:], -40.0)

            def softmax_front(b):
                # exp(l - 40): softmax-invariant shift keeps exp and 1/Z
                # in fp32 range.  Z + 1/Z on DVE, x/Z on Pool.
                nc.scalar.activation(
                    out=pexp[:, b], in_=logits_ps[b // 2][:, b % 2],
                    func=Exp, bias=shift[:],
                )
                nc.vector.reduce_sum(out=zsum[:, b], in_=pexp[:, b], axis=X)
                nc.vector.reciprocal(out=rinv[:, b], in_=zsum[:, b])
                nc.gpsimd.tensor_mul(
                    out=xr[:, b],
                    in0=x_sb[:, b],
                    in1=rinv[:, b].unsqueeze(-1).broadcast_to((128, C, I_LEN)),
                )

            def s_matmuls(b):
                t, b2 = divmod(b, 2)
                for c in range(C):
                    nc.tensor.matmul(
                        out=s_ps[t][b2 * 64 : (b2 + 1) * 64, :],
                        lhsT=pexp[:, b, c, :],
                        rhs=xr[:, b, c, :],
                        start=(c == 0),
                        stop=(c == C - 1),
                        tile_position=(0, 64 * b2),
                    )

            def gs_path(t):
                # gs[o,:] = G[o] @ s[o]; ns = dot(s, gs) = ||W s||^2.
                # Pair 1's muls run on Pool so the scheduler cannot slot
                # them into pair 0's DVE latency chain.
                eng = nc.vector if t == 0 else nc.gpsimd
                eng.tensor_mul(
                    out=gprod[:, t],
                    in0=g_sb[:],
                    in1=s_sb[:, t, :].unsqueeze(1).broadcast_to((128, I_LEN, I_LEN)),
                )
                nc.vector.reduce_sum(out=gs[:, t, :], in_=gprod[:, t], axis=X)
                eng.tensor_mul(out=dotp[:, t], in0=s_sb[:, t, :], in1=gs[:, t, :])
                nc.vector.reduce_sum(
                    out=ns[:, t : t + 1], in_=dotp[:, t].unsqueeze(1), axis=X
                )
                nc.vector.tensor_scalar_add(
                    out=denom[:, t : t + 1], in0=ns[:, t : t + 1], scalar1=1.0
                )
                nc.vector.reciprocal(out=rden[:, t : t + 1], in_=denom[:, t : t + 1])

            def squash_scalars(t):
                # factor = sqrt(ns)/(1+ns); sqrt via exp(0.5*ln) to stay
                # in one ACT table set.  (1+ns and its reciprocal are
                # emitted with the dot-reduce / final Square.)
                tsl = slice(t, t + 1)
                nc.scalar.activation(out=lnns[:, tsl], in_=ns[:, tsl], func=Ln)
                nc.scalar.activation(
                    out=vnorm[:, tsl], in_=lnns[:, tsl], func=Exp, scale=0.5
                )

            def wv_and_transpose(t, r):
                # wv = factor * gs for THIS iteration; the delta matmuls
                # accumulate logits across iterations in PSUM.
                tsl = slice(t, t + 1)
                nc.gpsimd.tensor_scalar(
                    out=wv_new[:, t],
                    in0=gs[:, t, :],
                    scalar1=vnorm[:, tsl],
                    scalar2=rden[:, tsl],
                    op0=MUL,
                    op1=MUL,
                )
                nc.tensor.transpose(
                    out=wvt_ps[t][:], in_=wv_new[:, t, :], identity=ident[:]
                )
                nc.scalar.copy(out=wvt_sb[:, t, :], in_=wvt_ps[t][:])

            def delta_matmuls(t, r):
                # logits[n,o] += sum_i x[n,i] wv_new[o,i] for half t
                # (batches 2t, 2t+1). r=0 opens each 2KB bank (start
                # lazily zeroes it); r=1 accumulates onto the surviving
                # has_written bits (group re-open: skip the sim's check).
                for b in (2 * t, 2 * t + 1):
                    b2 = b % 2
                    for c in range(C):
                        k = b2 * C + c
                        nc.tensor.matmul(
                            out=logits_ps[t][:, b2, c, :],
                            lhsT=xt_sb[:, b, c, :],
                            rhs=wvt_sb[:, t, b2 * 64 : (b2 + 1) * 64],
                            start=(r == 0 and k % 8 == 0),
                            stop=(r == 0 and (k % 8 == 7 or k == 2 * C - 1)),
                            skip_group_check=(r == 1),
                        )

            def final_pair(t):
                # r=2: v = factor * (W_li s) the direct way, then DMA out.
                tsl = slice(t, t + 1)
                (nc.vector if t == 0 else nc.gpsimd).tensor_mul(
                    out=prod[:, t],
                    in0=wli_sb[:],
                    in1=s_sb[:, t, :].unsqueeze(1).broadcast_to((128, L_LEN, I_LEN)),
                )
                nc.vector.reduce_sum(out=v_raw[:, t, :], in_=prod[:, t], axis=X)
                nc.scalar.activation(
                    out=sq[:, t], in_=v_raw[:, t], func=Square,
                    accum_out=ns[:, t : t + 1],
                )
                nc.vector.tensor_scalar_add(
                    out=denom[:, tsl], in0=ns[:, tsl], scalar1=1.0
                )
                nc.vector.reciprocal(out=rden[:, tsl], in_=denom[:, tsl])
                squash_scalars(t)
                nc.vector.scalar_tensor_tensor(
                    out=v[:, t],
                    in0=v_raw[:, t],
                    scalar=vnorm[:, tsl],
                    in1=rden[:, tsl].broadcast_to((128, L_LEN)),
                    op0=MUL,
                    op1=MUL,
                )
                nc.sync.dma_start(out=out_d[t], in_=v[:, t, :])

            # iteration 0: s is preloaded (uniform probs), no softmax.
            # Pair 1 trails pair 0 by design: wait-hints keep its heavy DVE
            # ops from being scheduled inside pair 0's latency chain.
            for t in range(PAIRS):
                gs_path(t)
                squash_scalars(t)
                wv_and_transpose(t, 0)
                delta_matmuls(t, 0)

            for r in (1, 2):
                for b in range(B):
                    softmax_front(b)
                for b in range(B):
                    s_matmuls(b)
                for t in range(PAIRS):
                    nc.scalar.copy(out=s_sb[:, t, :], in_=s_ps[t][:])
                    if r == 1:
                        gs_path(t)
                        squash_scalars(t)
                        wv_and_transpose(t, r)
                        delta_matmuls(t, r)
                    else:
                        final_pair(t)
    return nc


_NC = None


def get_nc():
    global _NC
    if _NC is None:
        _NC = build_nc()
    return _NC


def to_bf16(a):
    import ml_dtypes

    return a.astype(ml_dtypes.bfloat16)


def make_in_maps(x, weight):
    x = np.ascontiguousarray(x, dtype=np.float32)
    w = np.ascontiguousarray(weight, dtype=np.float32)
    g = np.einsum("oli,olj->oij", w, w).astype(np.float32)  # [64, 32, 32]
    g2 = np.tile(g, (2, 1, 1))  # pair-replicated [128, 32, 32]
    w2 = w.reshape(O_CAPS, L_LEN, I_LEN)
    ident = np.eye(128, dtype=np.float32)
    in_maps = []
    for core in range(NCORES):
        xs = x[core * B : (core + 1) * B]  # [B, 1152, 32]
        xc = xs.reshape(B, C, 128, I_LEN)
        x_nat = np.ascontiguousarray(xc.transpose(2, 0, 1, 3))  # [128, B, C, 32]
        xt = np.ascontiguousarray(xc.transpose(3, 0, 1, 2))  # [32, B, C, 128]
        # iter-0 s under uniform probs: s0[b] = sum_n x[b,n,:] / 64,
        # identical for every output capsule -> broadcast across partitions.
        s0b = xs.sum(axis=1) / O_CAPS  # [B, 32]
        s0 = np.empty((128, PAIRS, I_LEN), dtype=np.float32)
        for t in range(PAIRS):
            s0[0:64, t] = s0b[2 * t]
            s0[64:128, t] = s0b[2 * t + 1]
        in_maps.append(
            {
                "x_nat": to_bf16(x_nat),
                "xt": to_bf16(xt),
                "g": to_bf16(g2),
                "w_li": to_bf16(w2),
                "s0": to_bf16(s0),
                "ident": ident,
            }
        )
    return in_maps


def assemble(results):
    outs = []
    for core in range(NCORES):
        o = results[core]["out"]  # [PAIRS, 128, 32] -> [4, 64, 32]
        outs.append(np.asarray(o, dtype=np.float32).reshape(B, O_CAPS, L_LEN))
    return np.concatenate(outs, axis=0)


def _pin_act_table_set(nc):
    """Make Exp/Ln/Square/Copy resolve to the single table set containing
    all of them so the kernel pays one ACT table load."""
    from concourse.hw_specs import get_activation_tables

    tabs = get_activation_tables(nc.m.arch)
    for name, funcs in tabs.items():
        if name != "natural_log_exp_and_others":
            funcs.discard(Exp)
            funcs.discard(Ln)
            funcs.discard(Square)
            funcs.discard(mybir.ActivationFunctionType.Copy)
            funcs.discard(mybir.ActivationFunctionType.Identity)


def run(x, weight, trace=False):
    nc = get_nc()
    if not nc.is_finalized():
        _pin_act_table_set(nc)
        nc.finalize()
    res = run_bass_kernel_spmd(nc, make_in_maps(x, weight), list(range(NCORES)), trace=trace)
    return assemble(res.results), res


def kernel(x, weight):
    out, _ = run(x, weight)
    return out


# revision 11
# speedup vs baseline: 1.2459x; 1.2459x over previous
"""CapsuleLinear (dynamic routing) Trainium2 kernel — Gram-matrix formulation.

Reference computes priors = einsum('oli,bni->bonl', W, x) then 3 routing
iterations. We never materialize priors. Per routing iteration r:
    probs[n,o] = softmax_o(logits[n,o])        exp on ACT, Z on DVE
    s[o,i]     = sum_n probs[n,o] x[n,i]       PE matmul (contract n)
    gs[o,i]    = G[o] @ s[o]    G = W^T W      DVE mul+reduce (block-diag)
    ns[o]      = dot(s[o], gs[o]) = ||W s||^2  DVE mul + ACT accum
    factor[o]  = sqrt(ns)/(1+ns)               ACT ln/exp + DVE
    wv[o,i]    = factor * gs[o,i]              DVE tensor_scalar (tiny)
    wv_cum    += wv;  logits = x @ wv_cum^T    PE matmuls, fresh PSUM group
Final (r=2): v = factor * (W_li s) computed the direct way.

The Gram trick kills the old wv-step broadcast-mul+reduce entirely:
wv = factor*gs reuses the gs already needed for ns, and the out-step
(W_li s) only runs once, at r=2.

Iter 0 shortcut: probs are uniform (logits=0), so s0 = sum_n(x)/64 is
computed on host (tiny reduction) and DMA'd straight into s_sb: iteration
0 runs gs -> squash -> wv -> delta with no softmax/s-matmul at all.

Scheduling: pair 0 (batches 0,1) is the latency-critical chain each
iteration — its delta matmuls (h0) feed the next iteration's first exp.
Pair 1 trails at a constant offset. Emission order per engine queue keeps
pair-0 dependencies first.

Sharding: data-parallel over batch N=32 -> 4 per core on 8 cores, weight
and Gram replicated.  No collectives.
"""

import os
import sys

for _p in ("/opt/trn_rl_repo",):
    if _p not in sys.path and os.path.isdir(_p):
        sys.path.insert(0, _p)

import numpy as np

import concourse.bacc as bacc
import concourse.bass as bass
import concourse.tile as tile
from concourse import mybir
from concourse.bass_utils import run_bass_kernel_spmd

N_TOT, N_CAPS, I_LEN = 32, 1152, 32
O_CAPS, L_LEN = 64, 32
NCORES = 8
B = N_TOT // NCORES  # 4 batches per core
C = N_CAPS // 128    # 9 chunks of 128 input capsules
PAIRS = B // 2
FP = mybir.dt.float32
BF = mybir.dt.bfloat16
Exp = mybir.ActivationFunctionType.Exp
Ln = mybir.ActivationFunctionType.Ln
Square = mybir.ActivationFunctionType.Square
Copy = mybir.ActivationFunctionType.Copy
X = mybir.AxisListType.X
MUL = mybir.AluOpType.mult


def build_nc():
    nc = bacc.Bacc("TRN2", target_bir_lowering=False, debug=True)
    x_nat_d = nc.dram_tensor("x_nat", [128, B, C, I_LEN], BF, kind="ExternalInput")
    xt_d = nc.dram_tensor("xt", [I_LEN, B, C, 128], BF, kind="ExternalInput")
    g_d = nc.dram_tensor("g", [128, I_LEN, I_LEN], BF, kind="ExternalInput")
    w_li_d = nc.dram_tensor("w_li", [64, L_LEN, I_LEN], BF, kind="ExternalInput")
    s0_d = nc.dram_tensor("s0", [128, PAIRS, I_LEN], BF, kind="ExternalInput")
    ident_d = nc.dram_tensor("ident", [128, 128], FP, kind="ExternalInput")
    out_d = nc.dram_tensor("out", [PAIRS, 128, L_LEN], FP, kind="ExternalOutput")

    with tile.TileContext(nc) as tc:
        with (
            tc.tile_pool(name="main", bufs=1) as pool,
            tc.tile_pool(name="psum", bufs=1, space="PSUM") as psum,
        ):
            x_sb = pool.tile([128, B, C, I_LEN], BF)
            xt_sb = pool.tile([I_LEN, B, C, 128], BF)
            g_sb = pool.tile([128, I_LEN, I_LEN], BF)
            wli_sb = pool.tile([128, L_LEN, I_LEN], BF)
            ident = pool.tile([128, 128], FP)
            shift = pool.tile([128, 1], FP)
            pexp = pool.tile([128, B, C, O_CAPS], BF)
            zsum = pool.tile([128, B, C], FP)
            rinv = pool.tile([128, B, C], FP)
            xr = pool.tile([128, B, C, I_LEN], BF)
            s_sb = pool.tile([128, PAIRS, I_LEN], BF)
            gprod = pool.tile([128, PAIRS, I_LEN, I_LEN], BF)
            gs = pool.tile([128, PAIRS, I_LEN], FP)
            dotp = pool.tile([128, PAIRS, I_LEN], FP)
            ns = pool.tile([128, PAIRS], FP)
            lnns = pool.tile([128, PAIRS], FP)
            vnorm = pool.tile([128, PAIRS], FP)
            denom = pool.tile([128, PAIRS], FP)
            rden = pool.tile([128, PAIRS], FP)
            wv_new = pool.tile([128, PAIRS, I_LEN], FP)
            wvt_sb = pool.tile([I_LEN, PAIRS, 128], BF)
            # r=2 final out-step
            prod = pool.tile([128, PAIRS, L_LEN, I_LEN], BF)
            v_raw = pool.tile([128, PAIRS, L_LEN], FP)
            sq = pool.tile([128, PAIRS, L_LEN], FP)
            v = pool.tile([128, PAIRS, L_LEN], FP)

            logits_ps = [
                psum.tile([128, 2, C, O_CAPS], FP, name=f"logits_ps{h}", tag=f"lg{h}")
                for h in range(2)
            ]
            # s (bytes 0..127) and wvT (bytes 512..1023) share a bank per pair
            u_ps = [
                psum.tile([128, 512], FP, name=f"u_ps{t}", tag=f"u_ps{t}")
                for t in range(PAIRS)
            ]
            s_ps = [u_ps[t][:, 0:I_LEN] for t in range(PAIRS)]
            wvt_ps = [u_ps[t][0:I_LEN, 128:256] for t in range(PAIRS)]

            # s0+g first on the sync queue (they unblock iteration 0's
            # gs-path), x third; xt/ident on the gpsimd queue; wli (only
            # needed at r=2) on the scalar queue.
            nc.sync.dma_start(out=s_sb[:], in_=s0_d[:])
            nc.sync.dma_start(out=g_sb[:], in_=g_d[:])
            with tc.tile_wait_until(0.0015):
                nc.gpsimd.dma_start(out=xt_sb[:], in_=xt_d[:])
                nc.gpsimd.dma_start(out=ident[:], in_=ident_d[:])
            with tc.tile_wait_until(0.002):
                nc.sync.dma_start(out=x_sb[:], in_=x_nat_d[:])
            with tc.tile_wait_until(0.003):
                nc.scalar.dma_start(out=wli_sb[0:64], in_=w_li_d[:])
                nc.scalar.dma_start(out=wli_sb[64:128], in_=wli_sb[0:64])
            nc.vector.memset(shift[:], -40.0)

            def softmax_front(b):
                # exp(l - 40): softmax-invariant shift keeps exp and 1/Z
                # in fp32 range.  Z + 1/Z on DVE, x/Z on Pool.
                nc.scalar.activation(
                    out=pexp[:, b], in_=logits_ps[b // 2][:, b % 2],
                    func=Exp, bias=shift[:],
                )
                nc.vector.reduce_sum(out=zsum[:, b], in_=pexp[:, b], axis=X)
                nc.vector.reciprocal(out=rinv[:, b], in_=zsum[:, b])
                nc.gpsimd.tensor_mul(
                    out=xr[:, b],
                    in0=x_sb[:, b],
                    in1=rinv[:, b].unsqueeze(-1).broadcast_to((128, C, I_LEN)),
                )

            def s_matmuls(b):
                t, b2 = divmod(b, 2)
                for c in range(C):
                    nc.tensor.matmul(
                        out=s_ps[t][b2 * 64 : (b2 + 1) * 64, :],
                        lhsT=pexp[:, b, c, :],
                        rhs=xr[:, b, c, :],
                        start=(c == 0),
                        stop=(c == C - 1),
                        tile_position=(0, 64 * b2),
                    )

            def gs_path(t):
                # gs[o,:] = G[o] @ s[o]; ns = dot(s, gs) = ||W s||^2.
                eng = nc.vector
                eng.tensor_mul(
                    out=gprod[:, t],
                    in0=g_sb[:],
                    in1=s_sb[:, t, :].unsqueeze(1).broadcast_to((128, I_LEN, I_LEN)),
                )
                nc.vector.reduce_sum(out=gs[:, t, :], in_=gprod[:, t], axis=X)
                eng.tensor_mul(out=dotp[:, t], in0=s_sb[:, t, :], in1=gs[:, t, :])
                nc.vector.reduce_sum(
                    out=ns[:, t : t + 1], in_=dotp[:, t].unsqueeze(1), axis=X
                )
                nc.vector.tensor_scalar_add(
                    out=denom[:, t : t + 1], in0=ns[:, t : t + 1], scalar1=1.0
                )
                nc.vector.reciprocal(out=rden[:, t : t + 1], in_=denom[:, t : t + 1])

            def squash_scalars(t):
                # factor = sqrt(ns)/(1+ns); sqrt via exp(0.5*ln) to stay
                # in one ACT table set.  (1+ns and its reciprocal are
                # emitted with the dot-reduce / final Square.)
                tsl = slice(t, t + 1)
                nc.scalar.activation(out=lnns[:, tsl], in_=ns[:, tsl], func=Ln)
                nc.scalar.activation(
                    out=vnorm[:, tsl], in_=lnns[:, tsl], func=Exp, scale=0.5
                )

            def wv_and_transpose(t, r):
                # wv = factor * gs for THIS iteration; the delta matmuls
                # accumulate logits across iterations in PSUM.
                tsl = slice(t, t + 1)
                nc.vector.tensor_scalar(
                    out=wv_new[:, t],
                    in0=gs[:, t, :],
                    scalar1=vnorm[:, tsl],
                    scalar2=rden[:, tsl],
                    op0=MUL,
                    op1=MUL,
                )
                nc.tensor.transpose(
                    out=wvt_ps[t][:], in_=wv_new[:, t, :], identity=ident[:]
                )
                nc.scalar.copy(out=wvt_sb[:, t, :], in_=wvt_ps[t][:])

            def delta_matmuls(t, r):
                # logits[n,o] += sum_i x[n,i] wv_new[o,i] for half t
                # (batches 2t, 2t+1). r=0 opens each 2KB bank (start
                # lazily zeroes it); r=1 accumulates onto the surviving
                # has_written bits (group re-open: skip the sim's check).
                for b in (2 * t, 2 * t + 1):
                    b2 = b % 2
                    for c in range(C):
                        k = b2 * C + c
                        nc.tensor.matmul(
                            out=logits_ps[t][:, b2, c, :],
                            lhsT=xt_sb[:, b, c, :],
                            rhs=wvt_sb[:, t, b2 * 64 : (b2 + 1) * 64],
                            start=(r == 0 and k % 8 == 0),
                            stop=(r == 0 and (k % 8 == 7 or k == 2 * C - 1)),
                            skip_group_check=(r == 1),
                        )

            def final_pair(t):
                # r=2: v = factor * (W_li s) the direct way, then DMA out.
                tsl = slice(t, t + 1)
                nc.vector.tensor_mul(
                    out=prod[:, t],
                    in0=wli_sb[:],
                    in1=s_sb[:, t, :].unsqueeze(1).broadcast_to((128, L_LEN, I_LEN)),
                )
                nc.vector.reduce_sum(out=v_raw[:, t, :], in_=prod[:, t], axis=X)
                nc.scalar.activation(
                    out=sq[:, t], in_=v_raw[:, t], func=Square,
                    accum_out=ns[:, t : t + 1],
                )
                nc.vector.tensor_scalar_add(
                    out=denom[:, tsl], in0=ns[:, tsl], scalar1=1.0
                )
                nc.vector.reciprocal(out=rden[:, tsl], in_=denom[:, tsl])
                squash_scalars(t)
                nc.vector.scalar_tensor_tensor(
                    out=v[:, t],
                    in0=v_raw[:, t],
                    scalar=vnorm[:, tsl],
                    in1=rden[:, tsl].broadcast_to((128, L_LEN)),
                    op0=MUL,
                    op1=MUL,
                )
                nc.sync.dma_start(out=out_d[t], in_=v[:, t, :])

            # iteration 0: s is preloaded (uniform probs), no softmax.
            # Pair 1 trails pair 0 by design: wait-hints keep its heavy DVE
            # ops from being scheduled inside pair 0's latency chain.
            for t in range(PAIRS):
                gs_path(t)
                squash_scalars(t)
                wv_and_transpose(t, 0)
                delta_matmuls(t, 0)

            for r in (1, 2):
                for b in range(B):
                    softmax_front(b)
                for b in range(B):
                    s_matmuls(b)
                for t in range(PAIRS):
                    nc.scalar.copy(out=s_sb[:, t, :], in_=s_ps[t][:])
                    if r == 1:
                        gs_path(t)
                        squash_scalars(t)
                        wv_and_transpose(t, r)
                        delta_matmuls(t, r)
                    else:
                        final_pair(t)
    return nc


_NC = None


def get_nc():
    global _NC
    if _NC is None:
        _NC = build_nc()
    return _NC


def to_bf16(a):
    import ml_dtypes

    return a.astype(ml_dtypes.bfloat16)


def make_in_maps(x, weight):
    x = np.ascontiguousarray(x, dtype=np.float32)
    w = np.ascontiguousarray(weight, dtype=np.float32)
    g = np.einsum("oli,olj->oij", w, w).astype(np.float32)  # [64, 32, 32]
    g2 = np.tile(g, (2, 1, 1))  # pair-replicated [128, 32, 32]
    w2 = w.reshape(O_CAPS, L_LEN, I_LEN)
    ident = np.eye(128, dtype=np.float32)
    in_maps = []
    for core in range(NCORES):
        xs = x[core * B : (core + 1) * B]  # [B, 1152, 32]
        xc = xs.reshape(B, C, 128, I_LEN)
        x_nat = np.ascontiguousarray(xc.transpose(2, 0, 1, 3))  # [128, B, C, 32]
        xt = np.ascontiguousarray(xc.transpose(3, 0, 1, 2))  # [32, B, C, 128]
        # iter-0 s under uniform probs: s0[b] = sum_n x[b,n,:] / 64,
        # identical for every output capsule -> broadcast across partitions.
        s0b = xs.sum(axis=1) / O_CAPS  # [B, 32]
        s0 = np.empty((128, PAIRS, I_LEN), dtype=np.float32)
        for t in range(PAIRS):
            s0[0:64, t] = s0b[2 * t]
            s0[64:128, t] = s0b[2 * t + 1]
        in_maps.append(
            {
                "x_nat": to_bf16(x_nat),
                "xt": to_bf16(xt),
                "g": to_bf16(g2),
                "w_li": to_bf16(w2),
                "s0": to_bf16(s0),
                "ident": ident,
            }
        )
    return in_maps


def assemble(results):
    outs = []
    for core in range(NCORES):
        o = results[core]["out"]  # [PAIRS, 128, 32] -> [4, 64, 32]
        outs.append(np.asarray(o, dtype=np.float32).reshape(B, O_CAPS, L_LEN))
    return np.concatenate(outs, axis=0)


def _pin_act_table_set(nc):
    """Make Exp/Ln/Square/Copy resolve to the single table set containing
    all of them so the kernel pays one ACT table load."""
    from concourse.hw_specs import get_activation_tables

    tabs = get_activation_tables(nc.m.arch)
    for name, funcs in tabs.items():
        if name != "natural_log_exp_and_others":
            funcs.discard(Exp)
            funcs.discard(Ln)
            funcs.discard(Square)
            funcs.discard(mybir.ActivationFunctionType.Copy)
            funcs.discard(mybir.ActivationFunctionType.Identity)


def run(x, weight, trace=False):
    nc = get_nc()
    if not nc.is_finalized():
        _pin_act_table_set(nc)
        nc.finalize()
    res = run_bass_kernel_spmd(nc, make_in_maps(x, weight), list(range(NCORES)), trace=trace)
    return assemble(res.results), res


def kernel(x, weight):
    out, _ = run(x, weight)
    return out
